# revision 20
# baseline (speedup 1.0000x reference)
"""Trainium2 Bass kernel for nn_AttentionHeadAdaptive (scatter_memory).

Contract: kernel(**inputs) takes the FULL unsharded inputs (as produced by
reference.setup_inputs) and returns the full (z, feat_out, freq_out, min_out)
tuple. Internally shards batch B=16 across 8 NeuronCores (2 batches/core),
runs one SPMD Bass/Tile kernel, and reassembles on the host.

Per-batch device pipeline:
  - feat = concat(feat_mem[b], cls) resident in SBUF as 33 row-tiles [128,512]
    (f32r dtype: PE streams fp32 at bf16 rate with ~1.5e-4 matmul rms error,
    verified sufficient for the score-argmin margins of this problem).
  - featT tiles produced just-in-time by PE transpose; two gated-attention
    matmuls vs Wv/Wu; tanh/sigmoid on ACT; gate on DVE; Ww-matmul -> t scores.
  - argmin(t) / argmax(min/freq) with first-index tie-breaking via
    negated-space reductions (DVE row-reduce + GPSIMD cross-partition reduce).
  - eviction: per-tile indirect DMA scatter of feat rows; the evicted row is
    routed to a trash row (index 4096) that the host slices off -- in-bounds
    on purpose, since out-of-bounds drops cost ~ms each on this runtime.
    freq/min outputs built with a shifted/unshifted select, written as int32.
    feat is re-read as raw fp32 before the scatter so feat_out is bit-exact
    (f32r-typed DMA transfers round their payload).
  - z = sum(sigmoid(t) * feat) via PE accumulation.
"""
import sys

sys.path.insert(0, "/opt/trn_rl_repo")

import numpy as np

B, L, D = 16, 196, 512
M = 4096
NCORES = 8
BPC = B // NCORES          # batches per core
NTILES = 33                # 32 feat tiles + 1 tail tile (cls row + padding)
NTOK = 128 * NTILES        # 4224 token slots (4097 real)
BIGIDX = 99999.0

_compiled = {}


def _tok_layout(arr_flat: np.ndarray) -> np.ndarray:
    """[NTOK] -> [128, NTILES] with token i at (i % 128, i // 128)."""
    return np.ascontiguousarray(arr_flat.reshape(NTILES, 128).T)


def _build():
    import os
    SKIP = set(os.environ.get("K_SKIP", "").split(","))
    import concourse.bacc as bacc
    import concourse.bass as bass
    import concourse.bass_isa as bass_isa
    import concourse.mybir as mybir
    from concourse.tile import TileContext

    f32 = mybir.dt.float32
    f32r = mybir.dt.float32r
    i32 = mybir.dt.int32
    AX = mybir.AxisListType
    OP = mybir.AluOpType
    AF = mybir.ActivationFunctionType

    nc = bacc.Bacc("TRN2", target_bir_lowering=False, debug=False)

    feat_in = [nc.dram_tensor(f"feat{b}", [M, D], f32r, kind="ExternalInput") for b in range(BPC)]
    tail_in = [nc.dram_tensor(f"tail{b}", [128, D], f32r, kind="ExternalInput") for b in range(BPC)]
    wv_in = nc.dram_tensor("wv", [128, 16, 128], f32r, kind="ExternalInput")
    wu_in = nc.dram_tensor("wu", [128, 16, 128], f32r, kind="ExternalInput")
    ww_in = nc.dram_tensor("ww", [128, 4], f32r, kind="ExternalInput")
    ident_in = nc.dram_tensor("ident", [128, 128], f32r, kind="ExternalInput")
    ones_in = nc.dram_tensor("ones", [1, 128], f32, kind="ExternalInput")
    # packed fp32 constants, see column map below
    NCONST = 33 * 4 + BPC * 5 * 33 + 4 + 4 + 1
    consts_in = nc.dram_tensor("consts", [128, NCONST], f32, kind="ExternalInput")

    feat_out = [nc.dram_tensor(f"featout{b}", [M + 1, D], f32r, kind="ExternalOutput") for b in range(BPC)]
    freq_out = [nc.dram_tensor(f"freqout{b}", [M], i32, kind="ExternalOutput") for b in range(BPC)]
    min_out = [nc.dram_tensor(f"minout{b}", [M], i32, kind="ExternalOutput") for b in range(BPC)]
    z_out = nc.dram_tensor("z", [BPC, D], f32, kind="ExternalOutput")
    dbg_out = nc.dram_tensor("dbg", [BPC, 8], f32, kind="ExternalOutput")

    # const column offsets
    C_IOTA = 0            # iota (token index, f32)
    C_IOTANB = 33         # -iota + BIGIDX
    C_TMASK = 66          # 0 for tokens 0..4096, -1e30 for pads (negated-space argmin mask)
    C_DROP = 99           # 0 for tokens 0..4096, 2*BIGIDX for pads (scatter drop)
    def C_FREQ(b):  return 132 + b * 165
    def C_FREQS(b): return 132 + b * 165 + 33
    def C_FDIFF(b): return 132 + b * 165 + 66
    def C_MIN(b):   return 132 + b * 165 + 99
    def C_MINS(b):  return 132 + b * 165 + 132
    C_BV = 132 + BPC * 165
    C_BU = C_BV + 4
    C_BW = C_BU + 4

    with TileContext(nc) as tc:
        with (
            tc.tile_pool(name="persist", bufs=1) as pp,
            tc.tile_pool(name="work", bufs=2) as pw,
            tc.tile_pool(name="small", bufs=2) as psm,
            tc.tile_pool(name="ps_tr", bufs=2, space="PSUM") as ps_tr,
            tc.tile_pool(name="ps_mm", bufs=4, space="PSUM") as ps_mm,
            tc.tile_pool(name="ps_acc", bufs=2, space="PSUM") as ps_acc,
            tc.tile_pool(name="dram", bufs=2, space="DRAM") as pd,
        ):
            wv = pp.tile([128, 16, 128], f32r, tag="wv")
            wu = pp.tile([128, 16, 128], f32r, tag="wu")
            ww = pp.tile([128, 4], f32r, tag="ww")
            ident = pp.tile([128, 128], f32r, tag="ident")
            ones = pp.tile([1, 128], f32, tag="ones")
            cst = pp.tile([128, NCONST], f32, tag="cst")
            nc.sync.dma_start(out=wv[:], in_=wv_in[:])
            nc.sync.dma_start(out=wu[:], in_=wu_in[:])
            nc.sync.dma_start(out=ww[:], in_=ww_in[:])
            nc.sync.dma_start(out=ident[:], in_=ident_in[:])
            nc.sync.dma_start(out=ones[:], in_=ones_in[:])
            nc.sync.dma_start(out=cst[:], in_=consts_in[:])

            iota = cst[:, C_IOTA:C_IOTA + 33]
            iotanb = cst[:, C_IOTANB:C_IOTANB + 33]
            tmask = cst[:, C_TMASK:C_TMASK + 33]
            dropm = cst[:, C_DROP:C_DROP + 33]

            feat_sb = [
                pp.tile([128, NTILES, D], f32r, name=f"featsb{b}", tag=f"feat{b}")
                for b in range(BPC)
            ]

            def cross_max(src33, tagsfx, negate=False):
                """[128,33] f32 -> [128,1] tile with the global max on every
                partition (DVE row reduce + gpsimd partition all-reduce)."""
                row = psm.tile([128, 1], f32, tag="row" + tagsfx)
                nc.vector.tensor_reduce(out=row[:], in_=src33[:], axis=AX.X, op=OP.max)
                allb = psm.tile([128, 1], f32, tag="all" + tagsfx)
                nc.gpsimd.partition_all_reduce(out_ap=allb[:], in_ap=row[:], channels=128,
                                               reduce_op=bass_isa.ReduceOp.max)
                if negate:
                    nc.vector.tensor_scalar_mul(out=allb[:], in0=allb[:], scalar1=-1.0)
                return allb

            def first_index_neg(mask33, tagsfx):
                """mask (0/1 f32) -> [128,1] = smallest masked token index (bcast)."""
                cand = psm.tile([128, 33], f32, tag="cand" + tagsfx)
                nc.vector.tensor_tensor(out=cand[:], in0=mask33[:], in1=iotanb, op=OP.mult)
                nc.vector.tensor_scalar_add(out=cand[:], in0=cand[:], scalar1=-BIGIDX)
                return cross_max(cand, "ci" + tagsfx, negate=True)

            for b in range(BPC):
                fsb = feat_sb[b]
                nc.sync.dma_start(
                    out=fsb[:, 0:32, :],
                    in_=feat_in[b][:].rearrange("(c p) d -> p c d", p=128),
                )
                nc.sync.dma_start(out=fsb[:, 32, :], in_=tail_in[b][:])

                t_dram = pd.tile([1, NTOK], f32, tag="tdram")

                # ---- heavy phase: scores t for all tokens ----
                nts = [(nt, 512) for nt in range(8)] + [(8, 128)]
                featT = {}

                def emit_transposes(nt, NT):
                    ft = pw.tile([128, 4, 512], f32r, tag="featT")
                    ntile = NT // 128
                    for kc in range(4):
                        tp = ps_tr.tile([128, 512], f32r, tag="tr")
                        for j in range(ntile):
                            c = 4 * nt + j
                            nc.tensor.transpose(
                                out=tp[:, j * 128:(j + 1) * 128],
                                in_=fsb[:, c, kc * 128:(kc + 1) * 128],
                                identity=ident[:],
                            )
                        if kc % 2 == 0:
                            nc.vector.tensor_copy(out=ft[:, kc, :NT], in_=tp[:, :NT])
                        else:
                            nc.scalar.copy(out=ft[:, kc, :NT], in_=tp[:, :NT])
                    featT[nt] = ft

                emit_transposes(*nts[0])
                for nt, NT in nts:
                    if nt + 1 < len(nts):
                        emit_transposes(*nts[nt + 1])
                    ft = featT.pop(nt)
                    gated = pw.tile([128, 4, 512], f32r, tag="gated")
                    for oc in range(4):
                        pv = ps_mm.tile([128, 512], f32, tag="mm")
                        for kc in range(4):
                            nc.tensor.matmul(
                                pv[:, :NT], lhsT=wv[:, kc * 4 + oc, :],
                                rhs=ft[:, kc, :NT], start=(kc == 0), stop=(kc == 3),
                            )
                        pu = ps_mm.tile([128, 512], f32, tag="mm")
                        for kc in range(4):
                            nc.tensor.matmul(
                                pu[:, :NT], lhsT=wu[:, kc * 4 + oc, :],
                                rhs=ft[:, kc, :NT], start=(kc == 0), stop=(kc == 3),
                            )
                        v = pw.tile([128, 512], f32, tag="v")
                        u = pw.tile([128, 512], f32, tag="u")
                        nc.scalar.activation(out=v[:, :NT], in_=pv[:, :NT], func=AF.Tanh,
                                             bias=cst[:, C_BV + oc:C_BV + oc + 1])
                        nc.scalar.activation(out=u[:, :NT], in_=pu[:, :NT], func=AF.Sigmoid,
                                             bias=cst[:, C_BU + oc:C_BU + oc + 1])
                        nc.vector.tensor_tensor(out=gated[:, oc, :NT], in0=v[:, :NT],
                                                in1=u[:, :NT], op=OP.mult)
                    pt = ps_acc.tile([1, 512], f32, tag="acc1")
                    for oc in range(4):
                        nc.tensor.matmul(
                            pt[:, :NT], lhsT=ww[:, oc:oc + 1], rhs=gated[:, oc, :NT],
                            start=(oc == 0), stop=(oc == 3),
                        )
                    tst = pw.tile([1, 512], f32, tag="tst")
                    nc.scalar.copy(out=tst[:, :NT], in_=pt[:, :NT])
                    nc.sync.dma_start(out=t_dram[:, nt * 512:nt * 512 + NT], in_=tst[:, :NT])

                # ---- small phase: indices ----
                if "small" in SKIP:
                    continue
                t_tok = psm.tile([128, 33], f32, tag="ttok")
                nc.sync.dma_start(
                    out=t_tok[:],
                    in_=t_dram[:].rearrange("a (c p) -> (a p) c", p=128),
                )
                tneg = psm.tile([128, 33], f32, tag="tneg")
                nc.vector.tensor_scalar_mul(out=tneg[:], in0=t_tok[:], scalar1=-1.0)
                nc.vector.tensor_tensor(out=tneg[:], in0=tneg[:], in1=tmask, op=OP.add)
                tminb = cross_max(tneg, "tm")              # [128,1] = -tmin (bcast)
                ismin = psm.tile([128, 33], f32, tag="ismin")
                nc.vector.tensor_tensor(out=ismin[:], in0=tneg[:],
                                        in1=tminb[:].to_broadcast([128, 33]), op=OP.is_equal)
                aidx_b = first_index_neg(ismin, "a")       # [128,1] = attn_idx (bcast)

                bumpU = psm.tile([128, 33], f32, tag="bumpU")
                nc.vector.tensor_tensor(out=bumpU[:], in0=iota,
                                        in1=aidx_b[:].to_broadcast([128, 33]), op=OP.is_equal)
                mnU = psm.tile([128, 33], f32, tag="mnU")
                nc.vector.tensor_tensor(out=mnU[:], in0=cst[:, C_MIN(b):C_MIN(b) + 33],
                                        in1=bumpU[:], op=OP.add)
                aidx1_b = psm.tile([128, 1], f32, tag="aidx1")
                nc.vector.tensor_scalar_add(out=aidx1_b[:], in0=aidx_b[:], scalar1=-1.0)
                bumpS = psm.tile([128, 33], f32, tag="bumpS")
                nc.vector.tensor_tensor(out=bumpS[:], in0=iota,
                                        in1=aidx1_b[:].to_broadcast([128, 33]), op=OP.is_equal)
                mnS = psm.tile([128, 33], f32, tag="mnS")
                nc.vector.tensor_tensor(out=mnS[:], in0=cst[:, C_MINS(b):C_MINS(b) + 33],
                                        in1=bumpS[:], op=OP.add)

                freq_t = cst[:, C_FREQ(b):C_FREQ(b) + 33]
                rmask = psm.tile([128, 33], f32, tag="rmask")
                nc.vector.tensor_scalar(out=rmask[:], in0=freq_t, scalar1=5.5,
                                        scalar2=None, op0=OP.is_gt)
                ratio = psm.tile([128, 33], f32, tag="ratio")
                rcp = psm.tile([128, 33], f32, tag="rcp")
                nc.vector.reciprocal(out=rcp[:], in_=freq_t)
                # one Newton step: r1 = r0*(2 - f*r0)
                nwt = psm.tile([128, 33], f32, tag="nwt")
                nc.vector.tensor_tensor(out=nwt[:], in0=freq_t, in1=rcp[:], op=OP.mult)
                nc.vector.tensor_scalar(out=nwt[:], in0=nwt[:], scalar1=-1.0, scalar2=2.0,
                                        op0=OP.mult, op1=OP.add)
                nc.vector.tensor_tensor(out=rcp[:], in0=rcp[:], in1=nwt[:], op=OP.mult)
                nc.vector.tensor_tensor(out=ratio[:], in0=mnU[:], in1=rcp[:], op=OP.mult)
                nc.vector.tensor_tensor(out=ratio[:], in0=ratio[:], in1=rmask[:], op=OP.mult)
                maxr_b = cross_max(ratio, "r")
                ismax = psm.tile([128, 33], f32, tag="ismax")
                nc.vector.tensor_tensor(out=ismax[:], in0=ratio[:],
                                        in1=maxr_b[:].to_broadcast([128, 33]), op=OP.is_equal)
                rm_b = first_index_neg(ismax, "r")         # [128,1] = rm_idx (bcast)

                gt = psm.tile([128, 33], f32, tag="gt")
                nc.vector.tensor_tensor(out=gt[:], in0=iota,
                                        in1=rm_b[:].to_broadcast([128, 33]), op=OP.is_gt)
                eqrm = psm.tile([128, 33], f32, tag="eqrm")
                nc.vector.tensor_tensor(out=eqrm[:], in0=iota,
                                        in1=rm_b[:].to_broadcast([128, 33]), op=OP.is_equal)
                idxf = psm.tile([128, 33], f32, tag="idxf")
                nc.vector.tensor_tensor(out=idxf[:], in0=iota, in1=gt[:], op=OP.subtract)
                # dropped row rm -> trash row M: idxf += eqrm * (M - idxf)
                trm = psm.tile([128, 33], f32, tag="nwt")
                nc.vector.tensor_scalar(out=trm[:], in0=idxf[:], scalar1=-1.0, scalar2=float(M),
                                        op0=OP.mult, op1=OP.add)
                nc.vector.tensor_tensor(out=trm[:], in0=trm[:], in1=eqrm[:], op=OP.mult)
                nc.vector.tensor_tensor(out=idxf[:], in0=idxf[:], in1=trm[:], op=OP.add)
                idx_i = psm.tile([128, 33], i32, tag="idxi")
                nc.vector.tensor_copy(out=idx_i[:], in_=idxf[:])

                lt = psm.tile([128, 33], f32, tag="lt")
                nc.vector.tensor_tensor(out=lt[:], in0=iota,
                                        in1=rm_b[:].to_broadcast([128, 33]), op=OP.is_lt)
                fsel = psm.tile([128, 33], f32, tag="fsel")
                nc.vector.tensor_tensor(out=fsel[:], in0=lt[:],
                                        in1=cst[:, C_FDIFF(b):C_FDIFF(b) + 33], op=OP.mult)
                nc.vector.tensor_tensor(out=fsel[:], in0=fsel[:],
                                        in1=cst[:, C_FREQS(b):C_FREQS(b) + 33], op=OP.add)
                mdiff = psm.tile([128, 33], f32, tag="mdiff")
                nc.vector.tensor_tensor(out=mdiff[:], in0=mnU[:], in1=mnS[:], op=OP.subtract)
                msel = psm.tile([128, 33], f32, tag="msel")
                nc.vector.tensor_tensor(out=msel[:], in0=lt[:], in1=mdiff[:], op=OP.mult)
                nc.vector.tensor_tensor(out=msel[:], in0=msel[:], in1=mnS[:], op=OP.add)
                fsel_i = psm.tile([128, 33], i32, tag="fseli")
                msel_i = psm.tile([128, 33], i32, tag="mseli")
                nc.vector.tensor_copy(out=fsel_i[:], in_=fsel[:])
                nc.vector.tensor_copy(out=msel_i[:], in_=msel[:])
                nc.sync.dma_start(
                    out=freq_out[b][:].rearrange("(c p) -> p c", p=128),
                    in_=fsel_i[:, 0:32],
                )
                nc.sync.dma_start(
                    out=min_out[b][:].rearrange("(c p) -> p c", p=128),
                    in_=msel_i[:, 0:32],
                )

                # ---- z ----
                if "z" in SKIP:
                    continue
                s_tok = psm.tile([128, 33], f32, tag="stok")
                nc.scalar.activation(out=s_tok[:], in_=t_tok[:], func=AF.Sigmoid,
                                     bias=cst[:, C_BW:C_BW + 1])
                s_r = psm.tile([128, 33], f32r, tag="sr")
                nc.vector.tensor_copy(out=s_r[:], in_=s_tok[:])
                pz = ps_acc.tile([1, 512], f32, tag="acc1")
                for c in range(NTILES):
                    nc.tensor.matmul(pz[:], lhsT=s_r[:, c:c + 1], rhs=fsb[:, c, :],
                                     start=(c == 0), stop=(c == NTILES - 1))
                z_sb = psm.tile([1, 512], f32, tag="zsb")
                nc.scalar.copy(out=z_sb[:], in_=pz[:])
                nc.sync.dma_start(out=z_out[b:b + 1, :], in_=z_sb[:])

                # ---- feat scatter ----
                if "scatter" in SKIP:
                    continue
                # reload feat as exact fp32 bits (the f32r load rounded it)
                # into a fresh tile that reuses the same pool slot once the
                # last f32r consumer (z-mm) releases feat_sb
                exact_sb = pp.tile([128, NTILES, D], f32, name=f"exact{b}", tag=f"feat{b}")
                nc.sync.dma_start(
                    out=exact_sb[:, 0:32, :],
                    in_=feat_in[b][:].bitcast(f32).rearrange("(c p) d -> p c d", p=128),
                )
                nc.sync.dma_start(out=exact_sb[:, 32, :], in_=tail_in[b][:].bitcast(f32))
                for c in range(NTILES):
                    nc.gpsimd.indirect_dma_start(
                        out=feat_out[b][:].bitcast(f32),
                        out_offset=bass.IndirectOffsetOnAxis(ap=idx_i[:, c:c + 1], axis=0),
                        in_=exact_sb[:, c, :],
                        in_offset=None,
                        bounds_check=M - 1,
                        oob_is_err=False,
                    )

                # ---- debug ----
                dbg_sb = psm.tile([1, 8], f32, tag="dbg")
                nc.vector.tensor_scalar_mul(out=dbg_sb[:, 0:1], in0=aidx_b[0:1, :], scalar1=-1.0)
                nc.vector.tensor_scalar_mul(out=dbg_sb[:, 1:2], in0=rm_b[0:1, :], scalar1=-1.0)
                nc.vector.tensor_copy(out=dbg_sb[:, 2:3], in_=tminb[0:1, :])
                nc.vector.tensor_copy(out=dbg_sb[:, 3:4], in_=maxr_b[0:1, :])
                nc.vector.tensor_copy(out=dbg_sb[:, 4:8], in_=dbg_sb[:, 4:8])
                nc.sync.dma_start(out=dbg_out[b:b + 1, :], in_=dbg_sb[:])

    nc.compile()
    return nc


def _get_nc():
    if "nc" not in _compiled:
        _compiled["nc"] = _build()
    return _compiled["nc"]


def _prep_core_inputs(x, feat_mem, freq_mem, min_mem, Wv, bv, Wu, bu, Ww, bw, core):
    i0 = core * BPC
    ins = {}
    iota_f = np.arange(NTOK, dtype=np.float32)
    NCONST = 33 * 4 + BPC * 5 * 33 + 4 + 4 + 1
    cst = np.zeros((128, NCONST), np.float32)
    cst[:, 0:33] = _tok_layout(iota_f)
    cst[:, 33:66] = _tok_layout(-iota_f + BIGIDX)
    tm = np.zeros(NTOK, np.float32)
    tm[M + 1:] = -1e30
    cst[:, 66:99] = _tok_layout(tm)
    dp = np.zeros(NTOK, np.float32)
    dp[M + 1:] = 2 * BIGIDX
    cst[:, 99:132] = _tok_layout(dp)
    for b in range(BPC):
        gb = i0 + b
        ins[f"feat{b}"] = np.ascontiguousarray(feat_mem[gb])
        tail = np.zeros((128, D), np.float32)
        tail[0] = x[gb, 0, :]
        ins[f"tail{b}"] = tail
        freq_new = np.zeros(M + 1, np.float32)
        freq_new[:M] = freq_mem[gb].astype(np.float32)
        freq_new += 1.0
        fpad = np.full(NTOK, 1.0, np.float32)
        fpad[:M + 1] = freq_new
        fshift = np.full(NTOK, 1.0, np.float32)
        fshift[:M] = freq_new[1:M + 1]
        mcat = np.zeros(M + 1, np.float32)
        mcat[:M] = min_mem[gb].astype(np.float32)
        mpad = np.zeros(NTOK, np.float32)
        mpad[:M + 1] = mcat
        mshift = np.zeros(NTOK, np.float32)
        mshift[:M] = mcat[1:M + 1]
        o = 132 + b * 165
        cst[:, o:o + 33] = _tok_layout(fpad)
        cst[:, o + 33:o + 66] = _tok_layout(fshift)
        cst[:, o + 66:o + 99] = _tok_layout(fpad - fshift)
        cst[:, o + 99:o + 132] = _tok_layout(mpad)
        cst[:, o + 132:o + 165] = _tok_layout(mshift)
    C_BV = 132 + BPC * 165
    cst[:, C_BV:C_BV + 4] = bv.reshape(4, 128).T
    cst[:, C_BV + 4:C_BV + 8] = bu.reshape(4, 128).T
    cst[:, C_BV + 8] = bw[0]
    ins["consts"] = cst
    # weights: wv[p, kc*4+oc, j] = Wv[kc*128+p, oc*128+j]
    wv4 = Wv.reshape(4, 128, 4, 128)           # [kc, p, oc, j]
    ins["wv"] = np.ascontiguousarray(wv4.transpose(1, 0, 2, 3).reshape(128, 16, 128))
    wu4 = Wu.reshape(4, 128, 4, 128)
    ins["wu"] = np.ascontiguousarray(wu4.transpose(1, 0, 2, 3).reshape(128, 16, 128))
    ins["ww"] = np.ascontiguousarray(Ww.reshape(4, 128).T)
    ins["ident"] = np.eye(128, dtype=np.float32)
    ins["ones"] = np.ones((1, 128), np.float32)
    return ins


def kernel(x, feat_mem, freq_mem, min_mem, Wv, bv, Wu, bu, Ww, bw, _return_debug=False):
    from concourse.bass_utils import run_bass_kernel_spmd

    x = np.asarray(x, np.float32)
    feat_mem = np.asarray(feat_mem, np.float32)
    freq_mem = np.asarray(freq_mem, np.int32)
    min_mem = np.asarray(min_mem, np.int32)
    Wv = np.asarray(Wv, np.float32); bv = np.asarray(bv, np.float32)
    Wu = np.asarray(Wu, np.float32); bu = np.asarray(bu, np.float32)
    Ww = np.asarray(Ww, np.float32); bw = np.asarray(bw, np.float32)

    nc = _get_nc()
    in_maps = [
        _prep_core_inputs(x, feat_mem, freq_mem, min_mem, Wv, bv, Wu, bu, Ww, bw, core)
        for core in range(NCORES)
    ]
    res = None
    for attempt in range(3):
        try:
            res = run_bass_kernel_spmd(nc, in_maps, list(range(NCORES)))
            break
        except Exception:
            if attempt == 2:
                raise
            import time as _time
            _time.sleep(30)
    _compiled["last_results"] = res

    z = np.zeros((B, D), np.float32)
    feat_o = np.zeros((B, M, D), np.float32)
    freq_o = np.zeros((B, M), np.int32)
    min_o = np.zeros((B, M), np.int32)
    dbg = np.zeros((B, 8), np.float32)
    for core in range(NCORES):
        r = res.results[core]
        for b in range(BPC):
            gb = core * BPC + b
            z[gb] = r["z"][b]
            feat_o[gb] = r[f"featout{b}"][:M]
            freq_o[gb] = r[f"freqout{b}"]
            min_o[gb] = r[f"minout{b}"]
            dbg[gb] = r["dbg"][b]
    if _return_debug:
        return (z, feat_o, freq_o, min_o), dbg
    return z, feat_o, freq_o, min_o
